# revision 2
# baseline (speedup 1.0000x reference)
"""Trainium2 Bass kernel for nn_BERT_KNNCL_35527969473209 (retrieval_knn).

Contract: kernel(**inputs) takes the FULL inputs (liner_q [128,768] f32,
feature_queue [65536,768] f32, label_q [128] int, label_queue [65536] int)
and returns the FULL output [640, 64513] f32, matching:

    q = l2norm(liner_q); cos = q @ feature_queue.T
    pos = top_k(cos, 5) -> [640,1]
    neg = sort_desc(where(label match, -inf, cos))[:, :64512], rows repeated 5x
    out = concat([pos, neg], -1) / 0.07

Strategy (SPMD over 8 NeuronCores, queue-dim sharded):
  host: l2norm+1/T fold into q^T; transpose per-core feature chunk;
        per-core penalty matrix (-1e38 at label matches).
  core c: S = q^T.T @ fqt_c  [128 x 8192] (PE, fp32)
          top8/row via DVE InstMax (pre-mask); S += pen;
          per-row descending bitonic sort of the 8192-chunk (DVE);
          AllToAll 16-row shards -> row-owner core;
          bitonic merge of 8 sorted runs (DVE + DMA relabel between
          cross-partition stages); top5 = max8 of gathered top8s;
          write the [80 x 64513] shard (5x row replication via DMA).
  host: concatenate the 8 shards.
"""

import sys

import numpy as np

for _p in ("/opt/trn_rl_repo", "/root/.axon_site/_ro/trn_rl_repo"):
    if _p not in sys.path:
        sys.path.append(_p)

import concourse.bass as bass  # noqa: E402
import concourse.tile as tile  # noqa: E402
from concourse import bacc, mybir  # noqa: E402
from concourse.bass_utils import run_bass_kernel_spmd  # noqa: E402

F32 = mybir.dt.float32
MAX = mybir.AluOpType.max
MIN = mybir.AluOpType.min
ADD = mybir.AluOpType.add

NCORES = 8
B = 128
NROW = B // NCORES
NLBL = 64
TOPK = 5
KC = 8192
H = 768
T_TEMP = 0.07


def _log2i(n):
    k = n.bit_length() - 1
    assert (1 << k) == n
    return k


def build_nc(KC=KC, H=H):
    K = KC * NCORES
    POS = K // NLBL
    LAST = KC - POS
    NEG = K - POS
    OUTC = NEG + 1
    HC = H // 128
    JC = KC // 512

    nc = bacc.Bacc("TRN2", target_bir_lowering=False, debug=False,
                   num_devices=NCORES)

    qT = nc.dram_tensor("qT", [H, B], F32, kind="ExternalInput")
    fqt = nc.dram_tensor("fqt", [H, KC], F32, kind="ExternalInput")
    pen = nc.dram_tensor("pen", [B, KC], F32, kind="ExternalInput")
    out = nc.dram_tensor("out", [NROW * TOPK, OUTC], F32, kind="ExternalOutput")

    with tile.TileContext(nc) as tc:
        with (
            tc.tile_pool(name="fq", bufs=12) as fpool,
            tc.tile_pool(name="psum", bufs=4, space="PSUM") as ppool,
            tc.tile_pool(name="dram", bufs=1, space="DRAM") as dpool,
        ):
            S = nc.alloc_sbuf_tensor("S", [128, KC], F32).ap()
            T = nc.alloc_sbuf_tensor("T", [128, KC], F32).ap()
            V64 = nc.alloc_sbuf_tensor("V64", [64, KC], F32).ap()
            Y64 = nc.alloc_sbuf_tensor("Y64", [64, KC], F32).ap()
            qt_sb = nc.alloc_sbuf_tensor("qt_sb", [128, H], F32).ap()
            top8 = nc.alloc_sbuf_tensor("top8", [128, 8], F32).ap()
            T8 = nc.alloc_sbuf_tensor("T8", [16, 64], F32).ap()
            pos8 = nc.alloc_sbuf_tensor("pos8", [16, 8], F32).ap()

            a2a_in = dpool.tile([B, KC + 8], F32, tag="a2a_in")
            a2a_out = dpool.tile([B, KC + 8], F32, tag="a2a_out")

            # ---- load q^T and penalty ----
            for hc in range(HC):
                nc.sync.dma_start(qt_sb[:, hc * 128:(hc + 1) * 128],
                                  qT[hc * 128:(hc + 1) * 128, :])
            nc.sync.dma_start(T[:], pen[:])

            # ---- matmul S = q @ F^T ----
            for jc in range(JC):
                ftiles = []
                for hc in range(HC):
                    ft = fpool.tile([128, 512], F32, tag="ft")
                    nc.sync.dma_start(
                        ft[:], fqt[hc * 128:(hc + 1) * 128,
                                   jc * 512:(jc + 1) * 512])
                    ftiles.append(ft)
                ps = ppool.tile([128, 512], F32, tag="ps")
                for hc in range(HC):
                    nc.tensor.matmul(ps[:], qt_sb[:, hc * 128:(hc + 1) * 128],
                                     ftiles[hc][:], start=(hc == 0),
                                     stop=(hc == HC - 1))
                nc.scalar.activation(S[:, jc * 512:(jc + 1) * 512], ps[:],
                                     mybir.ActivationFunctionType.Copy)

            # ---- top8 per row (pre-mask) ----
            nc.vector.max(top8[:], S[:])

            # ---- mask positives ----
            nc.vector.tensor_tensor(S[:], S[:], T[:], ADD)

            # ---- per-row descending bitonic sort of the chunk ----
            cur, oth = S, T

            def halving(s):
                nonlocal cur, oth
                a = cur.rearrange("p (b two s) -> p b two s", two=2, s=s)
                o = oth.rearrange("p (b two s) -> p b two s", two=2, s=s)
                nc.vector.tensor_tensor(o[:, :, 0, :], a[:, :, 0, :],
                                        a[:, :, 1, :], MAX)
                nc.vector.tensor_tensor(o[:, :, 1, :], a[:, :, 0, :],
                                        a[:, :, 1, :], MIN)
                cur, oth = oth, cur

            for k in range(1, _log2i(KC) + 1):
                m = 1 << k
                a = cur.rearrange("p (b m) -> p b m", m=m)
                o = oth.rearrange("p (b m) -> p b m", m=m)
                lo = a[:, :, 0:m // 2]
                hi = a[:, :, m // 2:m]
                nc.vector.tensor_tensor(o[:, :, 0:m // 2], lo,
                                        hi[:, :, ::-1], MAX)
                nc.vector.tensor_tensor(o[:, :, m // 2:m], hi,
                                        lo[:, :, ::-1], MIN)
                cur, oth = oth, cur
                s = m // 4
                while s >= 1:
                    halving(s)
                    s //= 2

            # ---- stage for A2A ----
            nc.gpsimd.dma_start(a2a_in[:, 0:KC], cur[:])
            nc.gpsimd.dma_start(a2a_in[:, KC:KC + 8], top8[:])

            # ---- AllToAll (16-row shards) ----
            nc.gpsimd.collective_compute(
                "AllToAll", mybir.AluOpType.bypass,
                replica_groups=[list(range(NCORES))],
                ins=[a2a_in.opt()], outs=[a2a_out.opt()])

            # ---- load merge tile + top8 gather ----
            pos = {}
            for c in range(NCORES):
                g = (c // 2) if c % 2 == 0 else 4 + c // 2
                pos[c] = g
                nc.gpsimd.dma_start(cur[g * 16:(g + 1) * 16, :],
                                    a2a_out[c * 16:(c + 1) * 16, 0:KC])
                nc.gpsimd.dma_start(T8[:, c * 8:(c + 1) * 8],
                                    a2a_out[c * 16:(c + 1) * 16, KC:KC + 8])

            nc.vector.max(pos8[:], T8[:])

            # ---- merge 8 sorted runs ----
            def cross(pairs, rev, skip_v=False):
                nonlocal cur, oth
                if not skip_v:
                    for i, (lc, uc) in enumerate(pairs):
                        nc.gpsimd.dma_start(
                            V64[i * 16:(i + 1) * 16, :],
                            cur[pos[lc] * 16:(pos[lc] + 1) * 16, :])
                    vin = V64
                else:
                    vin = cur[0:64, :]
                for i, (lc, uc) in enumerate(pairs):
                    nc.gpsimd.dma_start(
                        Y64[i * 16:(i + 1) * 16, :],
                        cur[pos[uc] * 16:(pos[uc] + 1) * 16, :])
                y = Y64[:, ::-1] if rev else Y64[:]
                v = vin[:, ::-1] if rev else vin[:]
                nc.vector.tensor_tensor(oth[0:64, :], vin[:], y, MAX)
                nc.vector.tensor_tensor(oth[64:128, :], Y64[:], v, MIN)
                for i, (lc, uc) in enumerate(pairs):
                    pos[lc] = i
                    pos[uc] = 4 + i
                cur, oth = oth, cur

            def free_stages():
                s = KC // 2
                while s >= 1:
                    halving(s)
                    s //= 2

            cross([(0, 1), (2, 3), (4, 5), (6, 7)], rev=True, skip_v=True)
            free_stages()
            cross([(0, 3), (1, 2), (4, 7), (5, 6)], rev=True)
            cross([(0, 1), (2, 3), (4, 5), (6, 7)], rev=False)
            free_stages()
            cross([(0, 7), (1, 6), (2, 5), (3, 4)], rev=True)
            cross([(0, 2), (1, 3), (4, 6), (5, 7)], rev=False)
            cross([(0, 1), (2, 3), (4, 5), (6, 7)], rev=False)
            free_stages()

            # ---- outputs ----
            grp_chunk = sorted(range(8), key=lambda c: pos[c])
            fin = cur
            R2 = out.ap().flatten().rearrange("(r x) -> r x", x=TOPK * OUTC)
            for t in range(TOPK):
                for g in range(8):
                    cg = grp_chunk[g]
                    L = KC if cg < 7 else LAST
                    dst = R2[:, t * OUTC + 1 + cg * KC:
                             t * OUTC + 1 + cg * KC + L]
                    src = fin[g * 16:(g + 1) * 16, 0:L]
                    eng = [nc.gpsimd, nc.sync, nc.scalar][(t * 8 + g) % 3]
                    eng.dma_start(dst, src)
                with nc.allow_non_contiguous_dma(reason="16 scattered f32"):
                    nc.sync.dma_start(R2[:, t * OUTC:t * OUTC + 1],
                                      pos8[:, t:t + 1])

    nc.compile()
    return nc


_NC_CACHE = {}


def _get_nc():
    if "nc" not in _NC_CACHE:
        _NC_CACHE["nc"] = build_nc()
    return _NC_CACHE["nc"]


def host_inputs(liner_q, feature_queue, label_q, label_queue, KC=KC,
                T_temp=T_TEMP):
    lq = np.asarray(liner_q, dtype=np.float32)
    fq = np.asarray(feature_queue, dtype=np.float32)
    lbq = np.asarray(label_q).reshape(-1)
    lbQ = np.asarray(label_queue).reshape(-1)
    nrm = np.sqrt((lq * lq).sum(axis=1, keepdims=True))
    q = (lq / nrm / np.float32(T_temp)).astype(np.float32)
    qT = np.ascontiguousarray(q.T)
    in_maps = []
    for c in range(NCORES):
        sl = slice(c * KC, (c + 1) * KC)
        fqt_c = np.ascontiguousarray(fq[sl, :].T)
        pen_c = np.where(lbq[:, None] == lbQ[None, sl], np.float32(-1e38),
                         np.float32(0.0)).astype(np.float32)
        in_maps.append({"qT": qT, "fqt": fqt_c, "pen": pen_c})
    return in_maps


def _get_runner():
    """Cached jitted SPMD executable (avoids re-trace/re-compile per call)."""
    if "runner" in _NC_CACHE:
        return _NC_CACHE["runner"]
    import jax
    from jax.sharding import Mesh, NamedSharding, PartitionSpec
    from jax.experimental.shard_map import shard_map
    from concourse import bass2jax

    nc = _get_nc()
    partition_name = (nc.partition_id_tensor.name
                      if nc.partition_id_tensor else None)
    in_names, out_names, out_avals, out_shapes = [], [], [], []
    for alloc in nc.m.functions[0].allocations:
        if not isinstance(alloc, mybir.MemoryLocationSet):
            continue
        name = alloc.memorylocations[0].name
        if alloc.kind == "ExternalInput":
            if name != partition_name:
                in_names.append(name)
        elif alloc.kind == "ExternalOutput":
            out_names.append(name)
            shape = tuple(alloc.tensor_shape)
            dtype = mybir.dt.np(alloc.dtype)
            out_avals.append(jax.core.ShapedArray(shape, dtype))
            out_shapes.append((shape, dtype))
    n_params = len(in_names)
    all_in = list(in_names) + list(out_names)
    if partition_name is not None:
        all_in.append(partition_name)

    def _body(*args):
        operands = list(args)
        if partition_name is not None:
            operands.append(bass2jax.partition_id_tensor())
        return tuple(bass2jax._bass_exec_p.bind(
            *operands, out_avals=tuple(out_avals), in_names=tuple(all_in),
            out_names=tuple(out_names), lowering_input_output_aliases=(),
            sim_require_finite=True, sim_require_nnan=True, nc=nc))

    devices = jax.devices()[:NCORES]
    mesh = Mesh(np.asarray(devices), ("core",))
    fn = jax.jit(
        shard_map(_body, mesh=mesh,
                  in_specs=(PartitionSpec("core"),) * (n_params + len(out_names)),
                  out_specs=(PartitionSpec("core"),) * len(out_names),
                  check_rep=False),
        keep_unused=True)
    sharding = NamedSharding(mesh, PartitionSpec("core"))

    def runner(in_maps):
        per_core = [[np.asarray(m[nm]) for nm in in_names] for m in in_maps]
        concat_in = [np.concatenate([per_core[c][i] for c in range(NCORES)],
                                    axis=0) for i in range(n_params)]
        dev_in = [jax.device_put(a, sharding) for a in concat_in]
        dev_zero = [jax.device_put(
            np.zeros((NCORES * s[0], *s[1:]), d), sharding)
            for (s, d) in out_shapes]
        outs = fn(*dev_in, *dev_zero)
        out0 = np.asarray(outs[0])
        return out0  # [NCORES*80, OUTC] already concatenated core-major

    _NC_CACHE["runner"] = runner
    return runner


def run(inputs, trace=False, **kw):
    """Reference-path runner (used by test.py; returns BassKernelResults)."""
    nc = _get_nc()
    in_maps = host_inputs(**inputs)
    res = run_bass_kernel_spmd(nc, in_maps, core_ids=list(range(NCORES)),
                               trace=trace, **kw)
    full = np.concatenate([r["out"] for r in res.results], axis=0)
    return full, res


def kernel(liner_q, feature_queue, label_q, label_queue):
    inputs = dict(liner_q=liner_q, feature_queue=feature_queue,
                  label_q=label_q, label_queue=label_queue)
    try:
        runner = _get_runner()
        return runner(host_inputs(**inputs))
    except Exception:
        full, _ = run(inputs)
        return full


# revision 3
# speedup vs baseline: 1048.1857x; 1048.1857x over previous
"""Trainium2 Bass kernel for nn_BERT_KNNCL_35527969473209 (retrieval_knn).

Contract: kernel(**inputs) takes the FULL inputs (liner_q [128,768] f32,
feature_queue [65536,768] f32, label_q [128] int, label_queue [65536] int)
and returns the FULL output [640, 64513] f32, matching:

    q = l2norm(liner_q); cos = q @ feature_queue.T
    pos = top_k(cos, 5) -> [640,1]
    neg = sort_desc(where(label match, -inf, cos))[:, :64512], rows repeated 5x
    out = concat([pos, neg], -1) / 0.07

Strategy (SPMD over 8 NeuronCores, queue-dim sharded):
  host: l2norm+1/T fold into q^T; transpose per-core feature chunk;
        per-core penalty matrix (-1e38 at label matches).
  core c: S = q^T.T @ fqt_c  [128 x 8192] (PE, fp32)
          top8/row via DVE InstMax (pre-mask); S += pen;
          per-row descending bitonic sort of the 8192-chunk (DVE);
          AllToAll 16-row shards -> row-owner core;
          bitonic merge of 8 sorted runs (DVE + DMA relabel between
          cross-partition stages); top5 = max8 of gathered top8s;
          write the [80 x 64513] shard (5x row replication via DMA).
  host: concatenate the 8 shards.
"""

import sys

import numpy as np

for _p in ("/opt/trn_rl_repo", "/root/.axon_site/_ro/trn_rl_repo"):
    if _p not in sys.path:
        sys.path.append(_p)

import concourse.bass as bass  # noqa: E402
import concourse.tile as tile  # noqa: E402
from concourse import bacc, mybir  # noqa: E402
from concourse.bass_utils import run_bass_kernel_spmd  # noqa: E402

F32 = mybir.dt.float32
MAX = mybir.AluOpType.max
MIN = mybir.AluOpType.min
ADD = mybir.AluOpType.add

NCORES = 8
B = 128
NROW = B // NCORES
NLBL = 64
TOPK = 5
KC = 8192
H = 768
T_TEMP = 0.07


def _log2i(n):
    k = n.bit_length() - 1
    assert (1 << k) == n
    return k


def build_nc(KC=KC, H=H):
    K = KC * NCORES
    POS = K // NLBL
    LAST = KC - POS
    NEG = K - POS
    OUTC = NEG + 1
    HC = H // 128
    JC = KC // 512

    nc = bacc.Bacc("TRN2", target_bir_lowering=False, debug=False,
                   num_devices=NCORES)

    qT = nc.dram_tensor("qT", [H, B], F32, kind="ExternalInput")
    fqt = nc.dram_tensor("fqt", [H, KC], F32, kind="ExternalInput")
    pen = nc.dram_tensor("pen", [B, KC], F32, kind="ExternalInput")
    out = nc.dram_tensor("out", [NROW * TOPK, OUTC], F32, kind="ExternalOutput")

    with tile.TileContext(nc) as tc:
        with (
            tc.tile_pool(name="fq", bufs=12) as fpool,
            tc.tile_pool(name="psum", bufs=4, space="PSUM") as ppool,
            tc.tile_pool(name="dram", bufs=1, space="DRAM") as dpool,
        ):
            S = nc.alloc_sbuf_tensor("S", [128, KC], F32).ap()
            T = nc.alloc_sbuf_tensor("T", [128, KC], F32).ap()
            V64 = nc.alloc_sbuf_tensor("V64", [64, KC], F32).ap()
            Y64 = nc.alloc_sbuf_tensor("Y64", [64, KC], F32).ap()
            qt_sb = nc.alloc_sbuf_tensor("qt_sb", [128, H], F32).ap()
            top8 = nc.alloc_sbuf_tensor("top8", [128, 8], F32).ap()
            T8 = nc.alloc_sbuf_tensor("T8", [16, 64], F32).ap()
            pos8 = nc.alloc_sbuf_tensor("pos8", [16, 8], F32).ap()

            a2a_in = dpool.tile([B, KC + 8], F32, tag="a2a_in")
            a2a_out = dpool.tile([B, KC + 8], F32, tag="a2a_out")

            # ---- load q^T and penalty ----
            for hc in range(HC):
                nc.sync.dma_start(qt_sb[:, hc * 128:(hc + 1) * 128],
                                  qT[hc * 128:(hc + 1) * 128, :])
            nc.sync.dma_start(T[:], pen[:])

            # ---- matmul S = q @ F^T ----
            for jc in range(JC):
                ftiles = []
                for hc in range(HC):
                    ft = fpool.tile([128, 512], F32, tag="ft")
                    nc.sync.dma_start(
                        ft[:], fqt[hc * 128:(hc + 1) * 128,
                                   jc * 512:(jc + 1) * 512])
                    ftiles.append(ft)
                ps = ppool.tile([128, 512], F32, tag="ps")
                for hc in range(HC):
                    nc.tensor.matmul(ps[:], qt_sb[:, hc * 128:(hc + 1) * 128],
                                     ftiles[hc][:], start=(hc == 0),
                                     stop=(hc == HC - 1))
                nc.scalar.activation(S[:, jc * 512:(jc + 1) * 512], ps[:],
                                     mybir.ActivationFunctionType.Copy)

            # ---- top8 per row (pre-mask) ----
            nc.vector.max(top8[:], S[:])

            # ---- mask positives ----
            nc.vector.tensor_tensor(S[:], S[:], T[:], ADD)

            # ---- per-row descending bitonic sort of the chunk ----
            cur, oth = S, T

            def halving(s):
                nonlocal cur, oth
                a = cur.rearrange("p (b two s) -> p b two s", two=2, s=s)
                o = oth.rearrange("p (b two s) -> p b two s", two=2, s=s)
                nc.vector.tensor_tensor(o[:, :, 0, :], a[:, :, 0, :],
                                        a[:, :, 1, :], MAX)
                nc.vector.tensor_tensor(o[:, :, 1, :], a[:, :, 0, :],
                                        a[:, :, 1, :], MIN)
                cur, oth = oth, cur

            for k in range(1, _log2i(KC) + 1):
                m = 1 << k
                a = cur.rearrange("p (b m) -> p b m", m=m)
                o = oth.rearrange("p (b m) -> p b m", m=m)
                lo = a[:, :, 0:m // 2]
                hi = a[:, :, m // 2:m]
                nc.vector.tensor_tensor(o[:, :, 0:m // 2], lo,
                                        hi[:, :, ::-1], MAX)
                nc.vector.tensor_tensor(o[:, :, m // 2:m], hi,
                                        lo[:, :, ::-1], MIN)
                cur, oth = oth, cur
                s = m // 4
                while s >= 1:
                    halving(s)
                    s //= 2

            # ---- stage for A2A ----
            nc.gpsimd.dma_start(a2a_in[:, 0:KC], cur[:])
            nc.gpsimd.dma_start(a2a_in[:, KC:KC + 8], top8[:])

            # ---- AllToAll (16-row shards) ----
            nc.gpsimd.collective_compute(
                "AllToAll", mybir.AluOpType.bypass,
                replica_groups=[list(range(NCORES))],
                ins=[a2a_in.opt()], outs=[a2a_out.opt()])

            # ---- load merge tile + top8 gather ----
            pos = {}
            for c in range(NCORES):
                g = (c // 2) if c % 2 == 0 else 4 + c // 2
                pos[c] = g
                nc.gpsimd.dma_start(cur[g * 16:(g + 1) * 16, :],
                                    a2a_out[c * 16:(c + 1) * 16, 0:KC])
                nc.gpsimd.dma_start(T8[:, c * 8:(c + 1) * 8],
                                    a2a_out[c * 16:(c + 1) * 16, KC:KC + 8])

            nc.vector.max(pos8[:], T8[:])

            # ---- merge 8 sorted runs ----
            def cross(pairs, rev, skip_v=False):
                nonlocal cur, oth
                if not skip_v:
                    for i, (lc, uc) in enumerate(pairs):
                        nc.gpsimd.dma_start(
                            V64[i * 16:(i + 1) * 16, :],
                            cur[pos[lc] * 16:(pos[lc] + 1) * 16, :])
                    vin = V64
                else:
                    vin = cur[0:64, :]
                for i, (lc, uc) in enumerate(pairs):
                    nc.gpsimd.dma_start(
                        Y64[i * 16:(i + 1) * 16, :],
                        cur[pos[uc] * 16:(pos[uc] + 1) * 16, :])
                y = Y64[:, ::-1] if rev else Y64[:]
                v = vin[:, ::-1] if rev else vin[:]
                nc.vector.tensor_tensor(oth[0:64, :], vin[:], y, MAX)
                nc.vector.tensor_tensor(oth[64:128, :], Y64[:], v, MIN)
                for i, (lc, uc) in enumerate(pairs):
                    pos[lc] = i
                    pos[uc] = 4 + i
                cur, oth = oth, cur

            def free_stages():
                s = KC // 2
                while s >= 1:
                    halving(s)
                    s //= 2

            cross([(0, 1), (2, 3), (4, 5), (6, 7)], rev=True, skip_v=True)
            free_stages()
            cross([(0, 3), (1, 2), (4, 7), (5, 6)], rev=True)
            cross([(0, 1), (2, 3), (4, 5), (6, 7)], rev=False)
            free_stages()
            cross([(0, 7), (1, 6), (2, 5), (3, 4)], rev=True)
            cross([(0, 2), (1, 3), (4, 6), (5, 7)], rev=False)
            cross([(0, 1), (2, 3), (4, 5), (6, 7)], rev=False)
            free_stages()

            # ---- outputs ----
            grp_chunk = sorted(range(8), key=lambda c: pos[c])
            fin = cur
            R2 = out.ap().flatten().rearrange("(r x) -> r x", x=TOPK * OUTC)
            for t in range(TOPK):
                for g in range(8):
                    cg = grp_chunk[g]
                    L = KC if cg < 7 else LAST
                    dst = R2[:, t * OUTC + 1 + cg * KC:
                             t * OUTC + 1 + cg * KC + L]
                    src = fin[g * 16:(g + 1) * 16, 0:L]
                    eng = [nc.gpsimd, nc.sync, nc.scalar][(t * 8 + g) % 3]
                    eng.dma_start(dst, src)
                with nc.allow_non_contiguous_dma(reason="16 scattered f32"):
                    nc.sync.dma_start(R2[:, t * OUTC:t * OUTC + 1],
                                      pos8[:, t:t + 1])

    nc.compile()
    return nc


_NC_CACHE = {}


def _get_nc():
    if "nc" not in _NC_CACHE:
        _NC_CACHE["nc"] = build_nc()
    return _NC_CACHE["nc"]


def host_inputs(liner_q, feature_queue, label_q, label_queue, KC=KC,
                T_temp=T_TEMP):
    lq = np.asarray(liner_q, dtype=np.float32)
    fq = np.asarray(feature_queue, dtype=np.float32)
    lbq = np.asarray(label_q).reshape(-1)
    lbQ = np.asarray(label_queue).reshape(-1)
    nrm = np.sqrt((lq * lq).sum(axis=1, keepdims=True))
    q = (lq / nrm / np.float32(T_temp)).astype(np.float32)
    qT = np.ascontiguousarray(q.T)
    in_maps = []
    for c in range(NCORES):
        sl = slice(c * KC, (c + 1) * KC)
        fqt_c = np.ascontiguousarray(fq[sl, :].T)
        pen_c = np.where(lbq[:, None] == lbQ[None, sl], np.float32(-1e38),
                         np.float32(0.0)).astype(np.float32)
        in_maps.append({"qT": qT, "fqt": fqt_c, "pen": pen_c})
    return in_maps


def _get_runner():
    """Cached jitted SPMD executable (avoids re-trace/re-compile per call)."""
    if "runner" in _NC_CACHE:
        return _NC_CACHE["runner"]
    import jax
    from jax.sharding import Mesh, NamedSharding, PartitionSpec
    from jax.experimental.shard_map import shard_map
    from concourse import bass2jax

    nc = _get_nc()
    partition_name = (nc.partition_id_tensor.name
                      if nc.partition_id_tensor else None)
    in_names, out_names, out_avals, out_shapes = [], [], [], []
    for alloc in nc.m.functions[0].allocations:
        if not isinstance(alloc, mybir.MemoryLocationSet):
            continue
        name = alloc.memorylocations[0].name
        if alloc.kind == "ExternalInput":
            if name != partition_name:
                in_names.append(name)
        elif alloc.kind == "ExternalOutput":
            out_names.append(name)
            shape = tuple(alloc.tensor_shape)
            dtype = mybir.dt.np(alloc.dtype)
            out_avals.append(jax.core.ShapedArray(shape, dtype))
            out_shapes.append((shape, dtype))
    n_params = len(in_names)
    all_in = list(in_names) + list(out_names)
    if partition_name is not None:
        all_in.append(partition_name)

    def _body(*args):
        operands = list(args)
        if partition_name is not None:
            operands.append(bass2jax.partition_id_tensor())
        return tuple(bass2jax._bass_exec_p.bind(
            *operands, out_avals=tuple(out_avals), in_names=tuple(all_in),
            out_names=tuple(out_names), lowering_input_output_aliases=(),
            sim_require_finite=True, sim_require_nnan=True, nc=nc))

    devices = jax.devices()[:NCORES]
    mesh = Mesh(np.asarray(devices), ("core",))
    fn = jax.jit(
        shard_map(_body, mesh=mesh,
                  in_specs=(PartitionSpec("core"),) * (n_params + len(out_names)),
                  out_specs=(PartitionSpec("core"),) * len(out_names),
                  check_rep=False),
        keep_unused=True)
    sharding = NamedSharding(mesh, PartitionSpec("core"))

    import jax.numpy as jnp
    _zeros = jax.jit(
        lambda: tuple(jnp.zeros((NCORES * s[0], *s[1:]), d)
                      for (s, d) in out_shapes),
        out_shardings=tuple(sharding for _ in out_shapes))

    def prepare(in_maps):
        per_core = [[np.asarray(m[nm]) for nm in in_names] for m in in_maps]
        concat_in = [np.concatenate([per_core[c][i] for c in range(NCORES)],
                                    axis=0) for i in range(n_params)]
        dev_in = [jax.device_put(a, sharding) for a in concat_in]
        return dev_in

    def execute(dev_in):
        return fn(*dev_in, *_zeros())

    def runner(in_maps):
        outs = execute(prepare(in_maps))
        return np.asarray(outs[0])  # [NCORES*80, OUTC], core-major

    runner.prepare = prepare
    runner.execute = execute
    _NC_CACHE["runner"] = runner
    return runner


def run(inputs, trace=False, **kw):
    """Reference-path runner (used by test.py; returns BassKernelResults)."""
    nc = _get_nc()
    in_maps = host_inputs(**inputs)
    res = run_bass_kernel_spmd(nc, in_maps, core_ids=list(range(NCORES)),
                               trace=trace, **kw)
    full = np.concatenate([r["out"] for r in res.results], axis=0)
    return full, res


def kernel(liner_q, feature_queue, label_q, label_queue):
    inputs = dict(liner_q=liner_q, feature_queue=feature_queue,
                  label_q=label_q, label_queue=label_queue)
    try:
        runner = _get_runner()
        return runner(host_inputs(**inputs))
    except Exception:
        full, _ = run(inputs)
        return full
